# revision 12
# baseline (speedup 1.0000x reference)
"""ACAM Trainium2 kernel: 8-core data-parallel over batch.

Feature-major (transposed) pipeline, Nb=512 batch tiles, bf16 matmuls,
BN folded into linears on host, step-0 constant-folded on host.
Self-contained: hardcodes shapes from the problem spec.
"""
import sys
for _p in ("/opt/trn_rl_repo",):
    if _p not in sys.path:
        sys.path.insert(0, _p)

import numpy as np
import ml_dtypes

import concourse.bass as bass
import concourse.tile as tile
from concourse import mybir
from concourse.bass_utils import run_bass_kernel_spmd

EPS = 1e-5
B, W, F = 65536, 7, 80
H, E, A = 128, 128, 256
NUM_STEPS = 7
NCORES = 8
BL = B // NCORES          # 8192 batch per core
NB = 512                  # batch tile (free dim)
NT = BL // NB             # 16 tiles
WF = W * F                # 560
NCH = 5                   # 112-row chunks of the wf axis
CH = WF // NCH            # 112

bf16 = mybir.dt.bfloat16
f32 = mybir.dt.float32
AF = mybir.ActivationFunctionType
ALU = mybir.AluOpType

_cache = {}

# ---------------- host-side folding ----------------

def _npf(x):
    return np.asarray(x, dtype=np.float32)


def _fold_bn(w, b, bn):
    s = _npf(bn["gamma"]) / np.sqrt(_npf(bn["var"]) + EPS)
    t = _npf(bn["beta"]) - _npf(bn["mean"]) * s
    return _npf(w) * s[:, None], _npf(b) * s + t


def _ln_np(x, g, b):
    mu = x.mean(-1, keepdims=True)
    var = ((x - mu) ** 2).mean(-1, keepdims=True)
    return (x - mu) / np.sqrt(var + EPS) * g + b


def _prepare(params):
    p = {}
    wa1, ba1 = _fold_bn(params["enc_att1"]["w"], params["enc_att1"]["b"], params["enc_att_bn1"])
    wa2, ba2 = _fold_bn(params["enc_att2"]["w"], params["enc_att2"]["b"], params["enc_att_bn2"])
    wi1, bi1 = _fold_bn(params["enc_inp1"]["w"], params["enc_inp1"]["b"], params["enc_inp_bn1"])
    wi2, bi2 = _fold_bn(params["enc_inp2"]["w"], params["enc_inp2"]["b"], params["enc_inp_bn2"])
    wd, bd = _fold_bn(params["dec"]["w"], params["dec"]["b"], params["dec_bn"])
    w1, b1 = _fold_bn(params["act1"]["w"], params["act1"]["b"], params["act1_bn"])
    w2, b2 = _fold_bn(params["act2"]["w"], params["act2"]["b"], params["act2_bn"])
    wc, bc = _npf(params["cls"]["w"]), _npf(params["cls"]["b"])

    w_ih, w_hh = _npf(params["w_ih"]), _npf(params["w_hh"])
    b_g = _npf(params["b_ih"]) + _npf(params["b_hh"])
    lng, lnb = _npf(params["ln"]["gamma"]), _npf(params["ln"]["beta"])

    w_hh_g = w_hh * lng[None, :]
    b_g_full = b_g + w_hh @ lnb

    sig = lambda z: 1.0 / (1.0 + np.exp(-z))
    # step 0 on host (att=0, h=c=0; no LN at step 0)
    att = np.zeros(W, np.float32)
    a = np.maximum(wa1 @ att + ba1, 0.0)
    a = wa2 @ a + ba2
    x = np.maximum(bi1, 0.0)
    x = wi2 @ x + bi2
    agg = np.maximum(a + x, 0.0)
    gates = w_ih @ agg + b_g
    i_g, f_g, g_g, o_g = np.split(gates, 4)
    c1 = sig(i_g) * np.tanh(g_g)
    h1 = sig(o_g) * np.tanh(c1)
    s = sig(wd @ h1 + bd)
    att1 = s / s.sum()

    h1_ln = _ln_np(h1[None], lng, lnb)[0]
    c1_ln = _ln_np(c1[None], lng, lnb)[0]

    att1_exp = np.repeat(att1, F)
    wi1_s1 = wi1 * att1_exp[None, :]

    a_1 = np.maximum(wa1 @ att1 + ba1, 0.0)
    a2_1 = wa2 @ a_1 + ba2
    bagg1 = bi2 + a2_1
    bagg = bi2 + ba2
    bg1 = b_g + w_hh @ h1_ln

    p.update(wa1=wa1, ba1=ba1, wa2=wa2, wi2=wi2, wi1=wi1, bx1=bi1,
             wd=wd, bd=bd, w1=w1, b1=b1, w2=w2, b2=b2, wc=wc, bc=bc,
             w_ih=w_ih, w_hh_g=w_hh_g, b_g_full=b_g_full,
             wi1_s1=wi1_s1, bagg1=bagg1, bagg=bagg, bg1=bg1,
             c1_ln=c1_ln)
    return p


def _to_np_tree(obj):
    if isinstance(obj, dict):
        return {k: _to_np_tree(v) for k, v in obj.items()}
    return np.asarray(obj)


def _bf(x):
    return np.ascontiguousarray(np.asarray(x, np.float32).astype(ml_dtypes.bfloat16))


def _f32c(x):
    return np.ascontiguousarray(np.asarray(x, np.float32))


def build_inputs(p):
    d = {}
    d["wa1T"] = _bf(p["wa1"].T)
    d["wa2T"] = _bf(p["wa2"].T)
    d["wi2T"] = _bf(p["wi2"].T)
    d["wi1T"] = _bf(p["wi1"].T.reshape(NCH, CH, 128).transpose(1, 0, 2))
    d["wi1s1T"] = _bf(p["wi1_s1"].T.reshape(NCH, CH, 128).transpose(1, 0, 2))
    d["wihT"] = _bf(p["w_ih"].T)
    d["whhT"] = _bf(p["w_hh_g"].T)
    d["wdT"] = _bf(p["wd"].T)
    d["w1T"] = _bf(p["w1"].T)
    d["w2T"] = _bf(p["w2"].T.reshape(2, 128, 256).transpose(1, 0, 2))
    d["wcT"] = _bf(p["wc"].T.reshape(2, 128, W).transpose(1, 0, 2))
    pat = np.zeros((W, WF), np.float32)
    for w in range(W):
        pat[w, w * F:(w + 1) * F] = 1.0
    d["pat"] = _bf(pat)
    d["ones128"] = _bf(np.ones((128, 1)))
    d["ones128f"] = _f32c(np.ones((128, 1)))
    d["ones1x128"] = _bf(np.ones((1, 128)))
    d["ones7"] = _bf(np.ones((W, 1)))
    d["ones1x7"] = _bf(np.ones((1, W)))
    d["bx1"] = _f32c(p["bx1"].reshape(128, 1))
    d["ba1"] = _f32c(p["ba1"].reshape(128, 1))
    d["bagg"] = _f32c(p["bagg"].reshape(128, 1))
    d["bagg1"] = _f32c(p["bagg1"].reshape(128, 1))
    d["bg"] = _f32c(p["b_g_full"].reshape(4, 128).T)
    d["bg1"] = _f32c(p["bg1"].reshape(4, 128).T)
    d["cln1"] = _f32c(p["c1_ln"].reshape(128, 1))
    d["bd"] = _f32c(p["bd"].reshape(W, 1))
    d["b1"] = _f32c(p["b1"].reshape(2, 128).T)
    d["b2"] = _f32c(p["b2"].reshape(2, 128).T)
    d["bc"] = _f32c(p["bc"].reshape(W, 1))
    d["epsc"] = _f32c(np.full((1, 1), EPS))
    return d


def packed_inputs(p):
    packs = pack_consts(build_inputs(p))
    arrays = {k: v[0] for k, v in packs.items()}
    offsets = {k: v[1] for k, v in packs.items()}
    return arrays, offsets


BF_NAMES = ["wa1T", "wa2T", "wi2T", "wi1T", "wi1s1T", "wihT", "whhT",
            "wdT", "w1T", "w2T", "wcT", "pat", "ones128", "ones1x128",
            "ones7", "ones1x7"]
F32_NAMES = ["ones128f", "bx1", "ba1", "bagg", "bagg1", "bg", "bg1",
             "cln1", "bd", "b1", "b2", "bc", "epsc"]


def _flat2d(a):
    a = np.asarray(a)
    if a.ndim == 2:
        return a
    return a.reshape(a.shape[0], -1)


def pack_consts(d):
    """Pack consts into one bf16 and one f32 [128, X] array; return packed + offsets."""
    packs = {}
    for names, dt, key in ((BF_NAMES, ml_dtypes.bfloat16, "cpackb"),
                           (F32_NAMES, np.float32, "cpackf")):
        off = 0
        offs = {}
        mats = []
        for n in names:
            a = _flat2d(d[n])
            p, w = a.shape
            buf = np.zeros((128, w), dt)
            buf[:p, :] = a.astype(dt)
            mats.append(buf)
            offs[n] = (off, p, w)
            off += w
        packs[key] = (np.ascontiguousarray(np.concatenate(mats, axis=1)), offs)
    return packs


def build_graph(wd_shapes, offsets):
    from contextlib import ExitStack
    nc = bass.Bass()
    aps = {}
    for name, (shape, dt) in wd_shapes.items():
        aps[name] = nc.declare_dram_parameter(name, list(shape), dt, isOutput=False)
    outT = nc.declare_dram_parameter("outT", [W, BL], f32, isOutput=True)

    with tile.TileContext(nc) as tc, ExitStack() as ctx:
        _emit(ctx, tc, nc, aps, outT, offsets)
    _split_multi_waits(nc)
    return nc


def _split_multi_waits(nc):
    """Walrus here accepts one inline sync-wait per instruction; hoist extras
    into preceding same-engine NoOps."""
    nid = [0]
    for bb in nc.main_func.blocks:
        new = []
        changed = False
        for inst in bb.instructions:
            si = inst.sync_info
            if si is not None and len(si.on_wait) > 1:
                waits = list(si.on_wait)
                for wextra in waits[:-1]:
                    nop = mybir.InstNoOp(name=f"WSPL-{nid[0]}", ins=[], outs=[])
                    nid[0] += 1
                    nop.engine = inst.engine
                    nop.sync_info = mybir.SyncInfo(on_wait=[wextra], on_update=[])
                    new.append(nop)
                inst.sync_info = mybir.SyncInfo(on_wait=[waits[-1]],
                                                on_update=list(si.on_update))
                changed = True
            new.append(inst)
        if changed:
            bb.instructions = new


class _CView:
    """name -> SBUF slice of the packed const tiles."""

    def __init__(self, tiles, offsets):
        self.tiles = tiles      # {"cpackb": tile, "cpackf": tile}
        self.offsets = offsets  # {"cpackb": {name: (off,p,w)}, ...}

    def __getitem__(self, name):
        for key, offs in self.offsets.items():
            if name in offs:
                off, p, w = offs[name]
                return self.tiles[key][0:p, off:off + w]
        raise KeyError(name)

    def sl(self, name, lo, hi, p=None):
        for key, offs in self.offsets.items():
            if name in offs:
                off, pp, w = offs[name]
                return self.tiles[key][0:(p or pp), off + lo:off + hi]
        raise KeyError(name)


def _emit(ctx, tc, nc, aps, outT, offsets):
    cpool = ctx.enter_context(tc.tile_pool(name="consts", bufs=1))
    tiles = {}
    for key in ("cpackb", "cpackf"):
        ap = aps[key]
        t = cpool.tile(list(ap.shape), ap.dtype, tag=key)
        nc.gpsimd.dma_start(t[:], ap[:])
        tiles[key] = t
    c = _CView(tiles, offsets)
    # prime ACT/DVE engine clocks past the const DMAs so downstream
    # instructions carry only their single compute-producer wait
    prime = cpool.tile([1, 4], f32, tag="prime")
    nc.scalar.copy(prime[0:1, 0:1], tiles["cpackf"][0:1, 0:1])
    nc.scalar.copy(prime[0:1, 1:2], tiles["cpackb"][0:1, 0:1])
    nc.vector.tensor_copy(prime[0:1, 2:3], tiles["cpackf"][0:1, 0:1])
    nc.vector.tensor_copy(prime[0:1, 3:4], tiles["cpackb"][0:1, 0:1])

    featT = aps["featT"]  # [560, BL] bf16

    fpool = ctx.enter_context(tc.tile_pool(name="feat", bufs=2))
    spool = ctx.enter_context(tc.tile_pool(name="state", bufs=2))
    wpool = ctx.enter_context(tc.tile_pool(name="work", bufs=3))
    rpool = ctx.enter_context(tc.tile_pool(name="rows", bufs=3))
    # PSUM: mm(2) + expbc(2) + g0..g3(1 each) = 8 banks exactly
    mpool = ctx.enter_context(tc.tile_pool(name="mpsum", bufs=2, space="PSUM"))
    epool = ctx.enter_context(tc.tile_pool(name="epsum", bufs=2, space="PSUM"))
    gpool = ctx.enter_context(tc.tile_pool(name="gpsum", bufs=1, space="PSUM"))

    for j in range(NT):
        b0 = j * NB
        fT = fpool.tile([CH, NCH, NB], bf16, tag="fT")
        fsrc = featT.rearrange("(c p) b -> p c b", c=NCH)
        nc.gpsimd.dma_start(fT[:], fsrc[:, :, b0:b0 + NB])

        # ---- device step 1 (ref step 1) ----
        x1p = mpool.tile([E, NB], f32, tag="mm")
        for ci in range(NCH):
            nc.tensor.matmul(x1p[:], c.sl("wi1s1T", ci * 128, (ci + 1) * 128),
                             fT[:, ci, :], start=(ci == 0), stop=(ci == NCH - 1))
        x = wpool.tile([E, NB], bf16, tag="x")
        nc.scalar.activation(x[:], x1p[:], AF.Relu, bias=c["bx1"][:, 0:1])
        aggp = mpool.tile([E, NB], f32, tag="mm")
        nc.tensor.matmul(aggp[:], c["wi2T"][:], x[:], start=True, stop=True)
        agg = wpool.tile([E, NB], bf16, tag="agg")
        nc.scalar.activation(agg[:], aggp[:], AF.Relu, bias=c["bagg1"][:, 0:1])

        h = spool.tile([H, NB], bf16, tag="h")
        cst = spool.tile([H, NB], f32, tag="c")
        _cell(nc, c, wpool, gpool, agg, None, h, cst, bias=c["bg1"],
              c_ln=None, c_ln_scalar=c["cln1"])

        # ---- device steps 2..6 ----
        for step in range(2, NUM_STEPS):
            dp = mpool.tile([W, NB], f32, tag="mm")
            nc.tensor.matmul(dp[:], c["wdT"][:], h[:], start=True, stop=True)
            s = rpool.tile([W, NB], bf16, tag="s")
            nc.scalar.activation(s[:], dp[:], AF.Sigmoid, bias=c["bd"][:, 0:1])
            zp = mpool.tile([1, NB], f32, tag="mm")
            nc.tensor.matmul(zp[:], c["ones7"][:], s[:], start=True, stop=True)
            rz = rpool.tile([1, NB], bf16, tag="rz")
            with nc.allow_low_precision(reason="bf16 recip for bcast-mm rhs"):
                nc.vector.reciprocal(rz[:], zp[:])
            rzb = mpool.tile([W, NB], f32, tag="mm")
            nc.tensor.matmul(rzb[:], c["ones1x7"][:], rz[:], start=True, stop=True)
            att = rpool.tile([W, NB], bf16, tag="att")
            nc.vector.tensor_mul(att[:], s[:], rzb[:])

            x1p = mpool.tile([E, NB], f32, tag="mm")
            for ci in range(NCH):
                ep = epool.tile([E, NB], f32, tag="exp")
                nc.tensor.matmul(ep[:CH, :], c["pat"][:, ci * CH:(ci + 1) * CH],
                                 att[:], start=True, stop=True)
                atd = wpool.tile([CH, NB], bf16, tag="atd")
                nc.vector.tensor_mul(atd[:], fT[:, ci, :], ep[:CH, :])
                nc.tensor.matmul(x1p[:], c.sl("wi1T", ci * 128, (ci + 1) * 128),
                                 atd[:], start=(ci == 0), stop=(ci == NCH - 1))
            x = wpool.tile([E, NB], bf16, tag="x")
            nc.scalar.activation(x[:], x1p[:], AF.Relu, bias=c["bx1"][:, 0:1])
            a1p = mpool.tile([E, NB], f32, tag="mm")
            nc.tensor.matmul(a1p[:], c["wa1T"][:], att[:], start=True, stop=True)
            a1 = wpool.tile([E, NB], bf16, tag="a1")
            nc.scalar.activation(a1[:], a1p[:], AF.Relu, bias=c["ba1"][:, 0:1])
            aggp = mpool.tile([E, NB], f32, tag="mm")
            nc.tensor.matmul(aggp[:], c["wa2T"][:], a1[:], start=True, stop=False)
            nc.tensor.matmul(aggp[:], c["wi2T"][:], x[:], start=False, stop=True)
            agg = wpool.tile([E, NB], bf16, tag="agg")
            nc.scalar.activation(agg[:], aggp[:], AF.Relu, bias=c["bagg"][:, 0:1])

            h_ln = _ln(nc, c, wpool, rpool, mpool, epool, h, out_dt=bf16)
            c_ln = _ln(nc, c, wpool, rpool, mpool, epool, cst, out_dt=f32)

            h = spool.tile([H, NB], bf16, tag="h")
            cst_new = spool.tile([H, NB], f32, tag="c")
            _cell(nc, c, wpool, gpool, agg, h_ln, h, cst_new,
                  bias=c["bg"], c_ln=c_ln, c_ln_scalar=None)
            cst = cst_new

        # ---- head ----
        o1 = wpool.tile([128, 2, NB], bf16, tag="o1")
        for mi in range(2):
            hp = mpool.tile([128, NB], f32, tag="mm")
            nc.tensor.matmul(hp[:], c["w1T"][:, mi * 128:(mi + 1) * 128], h[:],
                             start=True, stop=True)
            nc.scalar.activation(o1[:, mi, :], hp[:], AF.Relu,
                                 bias=c["b1"][:, mi:mi + 1])
        o2 = wpool.tile([128, 2, NB], bf16, tag="o2")
        for mi in range(2):
            hp = mpool.tile([128, NB], f32, tag="mm")
            for ki in range(2):
                nc.tensor.matmul(hp[:], c.sl("w2T", ki * 256 + mi * 128, ki * 256 + (mi + 1) * 128),
                                 o1[:, ki, :], start=(ki == 0), stop=(ki == 1))
            nc.scalar.activation(o2[:, mi, :], hp[:], AF.Relu,
                                 bias=c["b2"][:, mi:mi + 1])
        lp = mpool.tile([W, NB], f32, tag="mm")
        for ki in range(2):
            nc.tensor.matmul(lp[:], c.sl("wcT", ki * W, (ki + 1) * W),
                             o2[:, ki, :], start=(ki == 0), stop=(ki == 1))
        res = rpool.tile([W, NB], f32, tag="res")
        nc.scalar.activation(res[:], lp[:], AF.Sigmoid, bias=c["bc"][:, 0:1])
        nc.gpsimd.dma_start(outT[:, b0:b0 + NB], res[:])


def _ln(nc, c, wpool, rpool, mpool, epool, xt, out_dt):
    """LayerNorm over partition dim (128) of xt [128, NB]; gamma/beta pre-folded."""
    NBl = xt.shape[-1]
    sq = wpool.tile([H, NBl], bf16, tag="sq")
    nc.scalar.activation(sq[:], xt[:], AF.Square)
    ones = c["ones128"] if xt.dtype == bf16 else c["ones128f"]
    s1 = mpool.tile([1, NBl], f32, tag="mm")
    nc.tensor.matmul(s1[:], ones[:], xt[:], start=True, stop=True)
    s2 = mpool.tile([1, NBl], f32, tag="mm")
    nc.tensor.matmul(s2[:], c["ones128"][:], sq[:], start=True, stop=True)
    musq = rpool.tile([1, NBl], f32, tag="musq")
    nc.scalar.activation(musq[:], s1[:], AF.Square, scale=1.0 / H)
    var = rpool.tile([1, NBl], f32, tag="var")
    nc.vector.scalar_tensor_tensor(var[:], s2[:], 1.0 / H, musq[:],
                                   ALU.mult, ALU.subtract)
    sd = rpool.tile([1, NBl], f32, tag="sd")
    nc.scalar.activation(sd[:], var[:], AF.Sqrt, bias=c["epsc"][0:1, 0:1])
    r = rpool.tile([1, NBl], bf16, tag="r")
    with nc.allow_low_precision(reason="bf16 recip for bcast-mm rhs"):
        nc.vector.reciprocal(r[:], sd[:])
    mu = rpool.tile([1, NBl], f32, tag="mu")
    nc.scalar.activation(mu[:], s1[:], AF.Copy, scale=1.0 / H)
    q = rpool.tile([1, NBl], bf16, tag="q")
    nc.vector.tensor_mul(q[:], mu[:], r[:])
    rb = epool.tile([H, NBl], f32, tag="exp")
    nc.tensor.matmul(rb[:], c["ones1x128"][:], r[:], start=True, stop=True)
    qb = epool.tile([H, NBl], f32, tag="exp")
    nc.tensor.matmul(qb[:], c["ones1x128"][:], q[:], start=True, stop=True)
    t = wpool.tile([H, NBl], f32, tag="lnt")
    nc.vector.tensor_mul(t[:], xt[:], rb[:])
    out = wpool.tile([H, NBl], out_dt, tag="lnout" + str(out_dt))
    nc.vector.tensor_sub(out[:], t[:], qb[:])
    return out


def _cell(nc, c, wpool, gpool, agg, h_ln, h_out, c_out, bias, c_ln, c_ln_scalar):
    NBl = agg.shape[-1]
    gp = []
    for mi in range(4):
        g = gpool.tile([H, NBl], f32, tag=f"g{mi}")
        nc.tensor.matmul(g[:], c["wihT"][:, mi * H:(mi + 1) * H], agg[:],
                         start=True, stop=(h_ln is None))
        if h_ln is not None:
            nc.tensor.matmul(g[:], c["whhT"][:, mi * H:(mi + 1) * H], h_ln[:],
                             start=False, stop=True)
        gp.append(g)
    ti = wpool.tile([H, NBl], f32, tag="ti")
    nc.scalar.activation(ti[:], gp[0][:], AF.Sigmoid, bias=bias[:, 0:1])
    tf = wpool.tile([H, NBl], f32, tag="tf")
    nc.scalar.activation(tf[:], gp[1][:], AF.Sigmoid, bias=bias[:, 1:2])
    tg = wpool.tile([H, NBl], f32, tag="tg")
    nc.scalar.activation(tg[:], gp[2][:], AF.Tanh, bias=bias[:, 2:3])
    to = wpool.tile([H, NBl], f32, tag="to")
    nc.scalar.activation(to[:], gp[3][:], AF.Sigmoid, bias=bias[:, 3:4])
    t1 = wpool.tile([H, NBl], f32, tag="t1")
    if c_ln_scalar is not None:
        nc.vector.tensor_scalar_mul(t1[:], tf[:], c_ln_scalar[:, 0:1])
    else:
        nc.vector.tensor_mul(t1[:], tf[:], c_ln[:])
    t2 = wpool.tile([H, NBl], f32, tag="t2")
    nc.vector.tensor_mul(t2[:], ti[:], tg[:])
    nc.vector.tensor_add(c_out[:], t1[:], t2[:])
    tc_ = wpool.tile([H, NBl], f32, tag="tcc")
    nc.scalar.activation(tc_[:], c_out[:], AF.Tanh)
    nc.vector.tensor_mul(h_out[:], to[:], tc_[:])


def kernel(features, params):
    feats = np.asarray(features, np.float32)
    p = _prepare(_to_np_tree(params))
    consts, offsets = packed_inputs(p)

    shards = []
    for ci in range(NCORES):
        fs = feats[ci * BL:(ci + 1) * BL]
        ft = np.ascontiguousarray(
            fs.reshape(BL, WF).T.astype(ml_dtypes.bfloat16))
        shards.append(ft)

    if "graph" not in _cache:
        wd_shapes = {n: (a.shape, bf16 if a.dtype == ml_dtypes.bfloat16 else f32)
                     for n, a in consts.items()}
        wd_shapes["featT"] = ((WF, BL), bf16)
        _cache["graph"] = build_graph(wd_shapes, offsets)
    nc = _cache["graph"]

    in_maps = []
    for ci in range(NCORES):
        m = dict(consts)
        m["featT"] = shards[ci]
        in_maps.append(m)

    res = run_bass_kernel_spmd(nc, in_maps, list(range(NCORES)))
    out = np.empty((B, W), np.float32)
    for ci in range(NCORES):
        out[ci * BL:(ci + 1) * BL] = np.asarray(res.results[ci]["outT"]).T
    return out
